# revision 1
# baseline (speedup 1.0000x reference)
"""Trainium2 Bass kernel for nn_Cross_Attention_Block_3624952397825.

Mathematical structure exploited: the reference takes ``out[:, -1, :]`` --
the attention output of the LAST query token. That token comes from the
zero row appended by ``jnp.pad`` AFTER the conv stack, so its query vector
is exactly zero, its attention scores are exactly zero, and softmax over
exact zeros is exactly uniform (1/4096).  Hence

    bins[b] = mean_k V[b, k, :] = (mean_k lidar[b, k, :]) @ wv
    out[b]  = MLP3(leaky_relu chain)(bins[b])

The conv block, Q/K projections, and softmax are structurally dead code
for ANY input values.  Additionally there is no nonlinearity between wv
and wo1, so W1 = wv @ wo1 [256, 128] is constant-folded on the host.

Per core (2 batches): stream lidar as fp16 [128, 4096] tiles (8 KiB per
partition -> full single-queue DMA rate), reduce the 4096 points with
ones^T @ tile matmuls on TensorE (fp16 x fp16 products are exact for a
1.0 stationary; accumulation is fp32 in PSUM), then a tiny fp16 MLP
(fp32 biases, fp32 final add).  Weights ride the second HWDGE queue
(ScalarE) so the lidar FIFO is never interrupted; batch 0 streams first
so its MLP overlaps batch 1's DMA.  Measured model error ~6e-4.
"""

import numpy as np

B, NPTS, CH, DM = 16, 4096, 256, 1024
N_CORES = 8
BL = B // N_CORES            # batches per core
P = 128
TILE_F = 4096                # free dim of lidar tiles (16 pts x 256 ch)
N_TILES = NPTS * CH // (P * TILE_F)   # 2 tiles per batch

# fp16 weight pack layout (free dim)
OFF_W1 = 0                   # 2 k-chunks x 128   (W1 = wv @ wo1)
OFF_WO2 = 256                # 128
OFF_WO3 = 384                # 256  (stored [K=128, 256] for row-form output)
OFF_ONE16 = 640              # fp16 ones column
W16_F = 641
# fp32 pack columns
C_B1, C_B2 = 0, 1
W32_F = 4

_CACHE = {}


def _build_program():
    import concourse.bacc as bacc
    import concourse.mybir as mybir
    from concourse.tile import TileContext

    f32 = mybir.dt.float32
    f16 = mybir.dt.float16
    Alu = mybir.AluOpType
    Act = mybir.ActivationFunctionType

    nc = bacc.Bacc("TRN2")
    lidar = nc.dram_tensor("lidar16", [BL, NPTS, CH], f16, kind="ExternalInput")
    wp16d = nc.dram_tensor("wp16", [P, W16_F], f16, kind="ExternalInput")
    wp32d = nc.dram_tensor("wp32", [P, W32_F], f32, kind="ExternalInput")
    b3rowd = nc.dram_tensor("b3row", [1, CH], f32, kind="ExternalInput")
    out_rows = nc.dram_tensor("out_rows", [BL, CH], f32, kind="ExternalOutput")

    # [BL, 4096, 256] -> [(b t), 128, 4096]; 8 KiB contiguous per partition.
    lv = lidar[:, :, :].rearrange("b (t p q) c -> (b t) p (q c)", p=P, q=16)

    with TileContext(nc) as tc:
        with (
            tc.tile_pool(name="w", bufs=1) as wpool,
            tc.tile_pool(name="io", bufs=4) as iopool,
            tc.tile_pool(name="small", bufs=1) as spool,
            tc.tile_pool(name="ps", bufs=2, space="PSUM") as pspool,
            tc.tile_pool(name="orp", bufs=2, space="PSUM") as orpool,
            tc.tile_pool(name="mm", bufs=3, space="PSUM") as mmpool,
        ):
            # weights on the ScalarE HWDGE queue; lidar owns the SP queue
            wp16 = wpool.tile([P, W16_F], f16, tag="wp16")
            nc.scalar.dma_start(out=wp16[:, :], in_=wp16d[:, :])
            wp32 = wpool.tile([P, W32_F], f32, tag="wp32")
            nc.scalar.dma_start(out=wp32[:, :], in_=wp32d[:, :])
            b3row = wpool.tile([1, CH], f32, tag="b3row")
            nc.scalar.dma_start(out=b3row[:, :], in_=b3rowd[:, :])
            ones16 = wp16[:, OFF_ONE16:OFF_ONE16 + 1]

            for b in range(BL):
                # ---- point reduction: ones^T @ tile on TensorE ----
                # fp16 x 1.0 products are exact; fp32 PSUM accumulation.
                # 512-wide moving operand (2 points x 256 ch per matmul).
                MM_F = 2 * CH
                sred = pspool.tile([1, MM_F], f32, tag="sred")
                nmm = N_TILES * (TILE_F // MM_F)
                i = 0
                for t in range(N_TILES):
                    tin = iopool.tile([P, TILE_F], f16, tag="tin")
                    nc.sync.dma_start(out=tin[:, :], in_=lv[b * N_TILES + t, :, :])
                    for j in range(TILE_F // MM_F):
                        nc.tensor.matmul(sred[:, :], lhsT=ones16,
                                         rhs=tin[:, j * MM_F:(j + 1) * MM_F],
                                         start=(i == 0), stop=(i == nmm - 1))
                        i += 1
                # fold [1, 512] -> fp16 [1, 256] sums via SBUF bounce
                s512 = spool.tile([1, MM_F], f32, tag=f"s512{b}")
                nc.scalar.copy(out=s512[:, :], in_=sred[:, :])
                s16 = spool.tile([1, CH], f16, tag=f"s16{b}")
                nc.vector.tensor_add(out=s16[:, :], in0=s512[0:1, 0:CH],
                                     in1=s512[0:1, CH:MM_F])
                # transpose row [1, 256] -> 2 x [128, 1] via K=1 fp16 matmuls;
                # mean scale (1/4096, exact power of two) folded into the copy
                mt = []
                for k in range(2):
                    mtp = mmpool.tile([P, 1], f32, tag="mm")
                    nc.tensor.matmul(mtp[:, :], lhsT=s16[0:1, k * P:(k + 1) * P],
                                     rhs=ones16[0:1, 0:1], start=True, stop=True)
                    mt16 = spool.tile([P, 1], f16, tag=f"mt{b}{k}")
                    nc.scalar.activation(mt16[:, :], mtp[:, :], Act.Copy,
                                         scale=float(1.0 / NPTS))
                    mt.append(mt16)

                def leaky(zp, bias_col, tag):
                    z = spool.tile([P, 1], f16, tag=f"z{tag}")
                    nc.scalar.activation(z[:, :], zp[:, :], Act.Identity,
                                         bias=wp32[:, bias_col:bias_col + 1], scale=1.0)
                    h = spool.tile([P, 1], f16, tag=f"h{tag}")
                    nc.vector.scalar_tensor_tensor(out=h[:, :], in0=z[:, :], scalar=0.01,
                                                   in1=z[:, :], op0=Alu.mult, op1=Alu.max)
                    return h

                # h1 = leaky(m @ W1 + b1), W1 pre-folded on host
                h1p = mmpool.tile([P, 1], f32, tag="mm")
                for k in range(2):
                    nc.tensor.matmul(h1p[:, :],
                                     lhsT=wp16[:, OFF_W1 + k * P: OFF_W1 + (k + 1) * P],
                                     rhs=mt[k][:, :], start=(k == 0), stop=(k == 1))
                h1 = leaky(h1p, C_B1, f"1{b}")

                h2p = mmpool.tile([P, 1], f32, tag="mm")
                nc.tensor.matmul(h2p[:, :], lhsT=wp16[:, OFF_WO2:OFF_WO2 + P],
                                 rhs=h1[:, :], start=True, stop=True)
                h2 = leaky(h2p, C_B2, f"2{b}")

                # final layer in row form: h2^T @ wo3 -> [1, 256]
                orp = orpool.tile([1, CH], f32, tag="orp")
                nc.tensor.matmul(orp[:, :], lhsT=h2[:, :],
                                 rhs=wp16[:, OFF_WO3:OFF_WO3 + CH],
                                 start=True, stop=True)
                orow = spool.tile([1, CH], f32, tag=f"orow{b}")
                nc.vector.tensor_add(out=orow[:, :], in0=orp[:, :], in1=b3row[:, :])
                nc.scalar.dma_start(out=out_rows[b:b + 1, :], in_=orow[:, :])

    nc.compile()
    return nc


def _pack_weights(inputs):
    wv = np.asarray(inputs["wv"], np.float64)
    wo1 = np.asarray(inputs["wo1"], np.float64)
    W1 = (wv @ wo1)                           # [256, 128], no nonlinearity between

    wp16 = np.zeros((P, W16_F), np.float16)
    wp16[:, OFF_W1:OFF_W1 + P] = W1[0:128, :]
    wp16[:, OFF_W1 + P:OFF_W1 + 2 * P] = W1[128:256, :]
    wp16[:, OFF_WO2:OFF_WO2 + P] = np.asarray(inputs["wo2"], np.float32)
    wp16[:, OFF_WO3:OFF_WO3 + CH] = np.asarray(inputs["wo3"], np.float32)
    wp16[:, OFF_ONE16] = 1.0

    wp32 = np.zeros((P, W32_F), np.float32)
    wp32[:, C_B1] = np.asarray(inputs["b1"], np.float32)
    wp32[:, C_B2] = np.asarray(inputs["b2"], np.float32)
    b3row = np.asarray(inputs["b3"], np.float32).reshape(1, CH)
    return wp16, wp32, b3row


def kernel(**inputs):
    from concourse.bass_utils import run_bass_kernel_spmd

    if "nc" not in _CACHE:
        _CACHE["nc"] = _build_program()
    nc = _CACHE["nc"]

    lidar16 = np.ascontiguousarray(
        np.asarray(inputs["lidar"], dtype=np.float32).astype(np.float16))
    wp16, wp32, b3row = _pack_weights(inputs)

    in_maps = [
        {"lidar16": lidar16[i * BL:(i + 1) * BL], "wp16": wp16,
         "wp32": wp32, "b3row": b3row}
        for i in range(N_CORES)
    ]
    res = run_bass_kernel_spmd(nc, in_maps, list(range(N_CORES)),
                               **_CACHE.get("run_kwargs", {}))
    _CACHE["last_results"] = res
    out = np.concatenate([res.results[i]["out_rows"] for i in range(N_CORES)], axis=0)
    return np.ascontiguousarray(out, dtype=np.float32)



# revision 6
# speedup vs baseline: 1.3297x; 1.3297x over previous
"""Trainium2 Bass kernel for nn_Cross_Attention_Block_3624952397825.

Mathematical structure exploited: the reference takes ``out[:, -1, :]`` --
the attention output of the LAST query token.  That token comes from the
zero row appended by ``jnp.pad`` AFTER the conv stack, so its query vector
is exactly zero, its attention scores are exactly zero, and softmax over
exact zeros is exactly uniform (1/4096).  Hence

    bins[b] = mean_k V[b, k, :] = (mean_k lidar[b, k, :]) @ wv
    out[b]  = MLP3(leaky_relu chain)(bins[b])

The conv block, Q/K projections, and softmax are structurally dead code
for ANY input values.  There is no nonlinearity between wv and wo1, so
W1 = wv @ wo1 [256, 128] is constant-folded on the host.

Kernel strategy (per core, 2 batches):
  * lidar is staged CHANNEL-MAJOR (host transpose) as fp8e3 (e3m4), which
    halves HBM traffic vs fp16; quantization error of the 4096-point mean
    stays ~1.2e-2 relative, under the 2e-2 gate (validated offline).
  * 4 tiles [128, 4096] stream on the sync HWDGE queue at ~320 GB/s.
  * The point-reduction is a FREE-DIM sum done by three engines in
    parallel on disjoint column ranges of each tile: DVE (reduce_sum),
    ACT (activation Copy with accum_out), GPSIMD (tensor_scalar with
    accum_out).  Combined ~480 G elem/s > DMA rate, so the kernel is
    DMA-bound with a short drain.
  * Channel-major partials are already the [128, 1] columns the MLP
    needs -- no fold/transpose stage.  Biases are added via K=1 rank-1
    matmuls (b ⊗ ones), keeping the tail on TensorE/DVE only.
  * Weights ride the scalar HWDGE queue; lidar owns the sync queue.
"""

import numpy as np

B, NPTS, CH, DM = 16, 4096, 256, 1024
N_CORES = 8
BL = B // N_CORES            # batches per core
P = 128
TILE_F = NPTS                # free dim of one (batch, channel-half) tile
N_TILES = BL * (CH // P)     # 4 tiles per core
# per-tile column split between the three reducers; GPSIMD pre-folds its
# range pairwise fp8+fp8 -> fp16 (exact), DVE then reduces the fp16 at 2x
DVE_N = 768
ACT_N = 928
GPS_N = TILE_F - DVE_N - ACT_N   # 2400, folded to 1200 fp16
# tile t = (batch b, half h) in order (b0,A),(b0,B),(b1,A),(b1,B);
# m-column layout wanted by the MLP: [Ab0, Ab1, Bb0, Bb1]
PERM = [0, 2, 1, 3]

# fp16 weight pack layout (free dim of wpack16 [128, 640])
OFF_W1A = 0      # W1[0:128, :]   (W1 = wv @ wo1)
OFF_W1B = 128    # W1[128:256, :]
OFF_WO2 = 256    # wo2 [128, 128]
OFF_WO3 = 384    # wo3 [128, 256]
W16_F = 640
# wrow16 [1, 258]: b1 row, b2 row, ones pair
OFF_B1, OFF_B2, OFF_ONES = 0, 128, 256
WROW_F = 258

_CACHE = {}


def _build_program():
    import concourse.bacc as bacc
    import concourse.mybir as mybir
    from concourse.tile import TileContext

    f32 = mybir.dt.float32
    f16 = mybir.dt.float16
    f8 = mybir.dt.float8e3
    Alu = mybir.AluOpType
    Act = mybir.ActivationFunctionType
    Ax = mybir.AxisListType

    nc = bacc.Bacc("TRN2")
    lidar8 = nc.dram_tensor("lidar8", [N_TILES, P, TILE_F], f8, kind="ExternalInput")
    wp16d = nc.dram_tensor("wp16", [P, W16_F], f16, kind="ExternalInput")
    wrowd = nc.dram_tensor("wrow", [1, WROW_F], f16, kind="ExternalInput")
    b3twod = nc.dram_tensor("b3two", [BL, CH], f32, kind="ExternalInput")
    out_rows = nc.dram_tensor("out_rows", [BL, CH], f32, kind="ExternalOutput")

    with TileContext(nc) as tc:
        with (
            tc.tile_pool(name="w", bufs=1) as wpool,
            tc.tile_pool(name="io", bufs=N_TILES) as iopool,
            tc.tile_pool(name="junk", bufs=2) as jpool,
            tc.tile_pool(name="small", bufs=1) as spool,
            tc.tile_pool(name="mm", bufs=2, space="PSUM") as mmpool,
            tc.tile_pool(name="orp", bufs=1, space="PSUM") as orpool,
        ):
            # weights on the scalar HWDGE queue; lidar owns the sync queue
            wp16 = wpool.tile([P, W16_F], f16, tag="wp16")
            nc.scalar.dma_start(out=wp16[:, :], in_=wp16d[:, :])
            wrow = wpool.tile([1, WROW_F], f16, tag="wrow")
            nc.scalar.dma_start(out=wrow[:, :], in_=wrowd[:, :])
            b3sb = wpool.tile([BL, CH], f32, tag="b3sb")
            nc.scalar.dma_start(out=b3sb[:, :], in_=b3twod[:, :])

            # partial sums: 3 engine-partials per tile, fp32
            S = spool.tile([P, 3 * N_TILES], f32, tag="S")
            GH = GPS_N // 2

            for t in range(N_TILES):
                tin = iopool.tile([P, TILE_F], f8, tag="tin")
                nc.sync.dma_start(out=tin[:, :], in_=lidar8[t, :, :])
                # DVE reduces its fp8 range directly
                nc.vector.reduce_sum(out=S[:, 3 * t:3 * t + 1],
                                     in_=tin[:, 0:DVE_N], axis=Ax.X)
                # ACT reduces via Copy-with-accumulator
                ja = jpool.tile([P, ACT_N], f16, tag="ja")
                nc.scalar.activation(ja[:, :], tin[:, DVE_N:DVE_N + ACT_N],
                                     Act.Copy,
                                     accum_out=S[:, 3 * t + 1:3 * t + 2])
                # GPSIMD folds fp8 pairs to fp16 (exact); DVE finishes at 2x
                jg = jpool.tile([P, GH], f16, tag="jg")
                nc.gpsimd.tensor_add(out=jg[:, :],
                                     in0=tin[:, DVE_N + ACT_N:DVE_N + ACT_N + GH],
                                     in1=tin[:, DVE_N + ACT_N + GH:TILE_F])
                nc.vector.reduce_sum(out=S[:, 3 * t + 2:3 * t + 3],
                                     in_=jg[:, :], axis=Ax.X)

            # fold the 3 partials per tile into the mean tile m16 [128, 4],
            # column order [Ab0, Ab1, Bb0, Bb1]; scale 1/4096 exact pow2
            m32 = spool.tile([P, N_TILES], f32, tag="m32")
            for t in range(N_TILES):
                nc.vector.reduce_sum(out=m32[:, PERM[t]:PERM[t] + 1],
                                     in_=S[:, 3 * t:3 * t + 3], axis=Ax.X)
            m16 = spool.tile([P, N_TILES], f16, tag="m16")
            nc.vector.tensor_scalar_mul(m16[:, :], m32[:, :], float(1.0 / NPTS))

            ones2 = wrow[0:1, OFF_ONES:OFF_ONES + BL]

            def leaky(zp, tag):
                z01 = spool.tile([P, BL], f16, tag=f"z{tag}")
                nc.vector.tensor_scalar_mul(z01[:, :], zp[:, :], 0.01)
                h = spool.tile([P, BL], f16, tag=f"h{tag}")
                nc.vector.tensor_max(h[:, :], zp[:, :], z01[:, :])
                return h

            # h1 = leaky(W1.T @ m + b1 ⊗ ones)
            h1p = mmpool.tile([P, BL], f32, tag="mm")
            nc.tensor.matmul(h1p[:, :], lhsT=wp16[:, OFF_W1A:OFF_W1A + P],
                             rhs=m16[:, 0:BL], start=True, stop=False)
            nc.tensor.matmul(h1p[:, :], lhsT=wp16[:, OFF_W1B:OFF_W1B + P],
                             rhs=m16[:, BL:2 * BL], start=False, stop=False)
            nc.tensor.matmul(h1p[:, :], lhsT=wrow[0:1, OFF_B1:OFF_B1 + P],
                             rhs=ones2, start=False, stop=True)
            h1 = leaky(h1p, "1")

            # h2 = leaky(wo2.T @ h1 + b2 ⊗ ones)
            h2p = mmpool.tile([P, BL], f32, tag="mm")
            nc.tensor.matmul(h2p[:, :], lhsT=wp16[:, OFF_WO2:OFF_WO2 + P],
                             rhs=h1[:, :], start=True, stop=False)
            nc.tensor.matmul(h2p[:, :], lhsT=wrow[0:1, OFF_B2:OFF_B2 + P],
                             rhs=ones2, start=False, stop=True)
            h2 = leaky(h2p, "2")

            # out rows = h2.T @ wo3 + b3  -> [2, 256]
            orp = orpool.tile([BL, CH], f32, tag="orp")
            nc.tensor.matmul(orp[:, :], lhsT=h2[:, :],
                             rhs=wp16[:, OFF_WO3:OFF_WO3 + CH],
                             start=True, stop=True)
            orow = spool.tile([BL, CH], f32, tag="orow")
            nc.vector.tensor_add(out=orow[:, :], in0=orp[:, :], in1=b3sb[:, :])
            nc.scalar.dma_start(out=out_rows[:, :], in_=orow[:, :])

    nc.compile()
    return nc


def _pack_weights(inputs):
    import ml_dtypes

    wv = np.asarray(inputs["wv"], np.float64)
    wo1 = np.asarray(inputs["wo1"], np.float64)
    W1 = (wv @ wo1)                            # [256, 128], linear chain

    wp16 = np.zeros((P, W16_F), np.float16)
    wp16[:, OFF_W1A:OFF_W1A + P] = W1[0:128, :]
    wp16[:, OFF_W1B:OFF_W1B + P] = W1[128:256, :]
    wp16[:, OFF_WO2:OFF_WO2 + P] = np.asarray(inputs["wo2"], np.float32)
    wp16[:, OFF_WO3:OFF_WO3 + CH] = np.asarray(inputs["wo3"], np.float32)

    wrow = np.zeros((1, WROW_F), np.float16)
    wrow[0, OFF_B1:OFF_B1 + P] = np.asarray(inputs["b1"], np.float32)
    wrow[0, OFF_B2:OFF_B2 + P] = np.asarray(inputs["b2"], np.float32)
    wrow[0, OFF_ONES:OFF_ONES + BL] = 1.0

    b3two = np.broadcast_to(
        np.asarray(inputs["b3"], np.float32).reshape(1, CH), (BL, CH)).copy()
    return wp16, wrow, b3two


def kernel(**inputs):
    import ml_dtypes
    from concourse.bass_utils import run_bass_kernel_spmd

    if "nc" not in _CACHE:
        _CACHE["nc"] = _build_program()
    nc = _CACHE["nc"]

    # channel-major fp8e3 staging: [16, 4096, 256] -> per-core
    # [2, 256, 4096] -> [4, 128, 4096]
    lid = np.asarray(inputs["lidar"], dtype=np.float32)
    lid8 = np.ascontiguousarray(lid.transpose(0, 2, 1)).astype(
        ml_dtypes.float8_e3m4).reshape(N_CORES, N_TILES, P, TILE_F)
    wp16, wrow, b3two = _pack_weights(inputs)

    in_maps = [
        {"lidar8": lid8[i], "wp16": wp16, "wrow": wrow, "b3two": b3two}
        for i in range(N_CORES)
    ]
    res = run_bass_kernel_spmd(nc, in_maps, list(range(N_CORES)),
                               **_CACHE.get("run_kwargs", {}))
    _CACHE["last_results"] = res
    out = np.concatenate([res.results[i]["out_rows"] for i in range(N_CORES)], axis=0)
    return np.ascontiguousarray(out, dtype=np.float32)


# revision 10
# speedup vs baseline: 1.4102x; 1.0606x over previous
"""Trainium2 Bass kernel for nn_Cross_Attention_Block_3624952397825.

Mathematical structure exploited: the reference takes ``out[:, -1, :]`` --
the attention output of the LAST query token.  That token comes from the
zero row appended by ``jnp.pad`` AFTER the conv stack, so its query vector
is exactly zero, its attention scores are exactly zero, and softmax over
exact zeros is exactly uniform (1/4096).  Hence

    bins[b] = mean_k V[b, k, :] = (mean_k lidar[b, k, :]) @ wv
    out[b]  = MLP3(leaky_relu chain)(bins[b])

The conv block, Q/K projections and softmax are structurally dead code for
ANY input values.  There is no nonlinearity between wv and wo1, so
W1 = wv @ wo1 [256, 128] is constant-folded on the host.

Kernel strategy (per core, 2 batches): lidar is quantized fp8e3 on the
host (~1.2e-2 rel err, under the 2e-2 gate; halves HBM bytes vs fp16) and
split per batch into
  * a POINT-MAJOR region (PTS_PM pts): reduced on TensorE by ones^T @ tile
    matmul chains (~0.6 ns/col in PSUM-accumulate chains), then folded
    [1,512]->[1,256] and transposed to columns via K=1 matmuls;
  * a CHANNEL-MAJOR region (host-transposed; PTS_CM pts): free-dim-reduced
    in parallel by DVE (reduce_sum), ACT (Copy + accum_out) and GPSIMD
    (pairwise fp8+fp8->fp16 fold, exact, re-reduced by DVE), with the
    split sized from measured rates (DVE 0.81 / ACT 1.2 / GPS 0.52
    elem/lane/ns).
Chunks stream on the sync HWDGE queue ordered pm_b0, cm_b0(2), pm_b1,
cm_b1(2) so every engine's feed arrives early and the last chunks are
small.  The MLP tail runs on TensorE/DVE with biases applied as K=1
rank-1 matmuls.
"""

import numpy as np

B, NPTS, CH, DM = 16, 4096, 256, 1024
N_CORES = 8
BL = B // N_CORES            # batches per core
P = 128

PTS_PM = 2048                # point-major points per batch (TensorE share)
PTS_CM = NPTS - PTS_PM       # 2048 channel-major points per batch
PM_F = PTS_PM * CH // P      # 4096 free dim of one pm tile
MM_F = 2 * CH                # 512-wide matmul slabs (2 pts x 256 ch)

# per-half split of the channel-major reduction (of PTS_CM columns)
DVE_N = 256                  # direct fp8 on DVE
ACT_N = 612
GPS_N = PTS_CM - DVE_N - ACT_N   # 1180 folded pairwise by GPSIMD
GH = GPS_N // 2              # 590 fp16 fold outputs re-reduced by DVE

# fp16 weight pack layout (free dim of wpack16 [128, 640])
OFF_W1A = 0      # W1[0:128, :]   (W1 = wv @ wo1)
OFF_W1B = 128    # W1[128:256, :]
OFF_WO2 = 256    # wo2 [128, 128]
OFF_WO3 = 384    # wo3 [128, 256]
W16_F = 640
# wrow16 [1, 260]: b1 row, b2 row, ones pair
OFF_B1, OFF_B2, OFF_ONES = 0, 128, 256
WROW_F = 260

_CACHE = {}


def _build_program():
    import concourse.bacc as bacc
    import concourse.mybir as mybir
    from concourse.tile import TileContext

    f32 = mybir.dt.float32
    f16 = mybir.dt.float16
    f8 = mybir.dt.float8e3
    Alu = mybir.AluOpType
    Act = mybir.ActivationFunctionType
    Ax = mybir.AxisListType

    nc = bacc.Bacc("TRN2")
    # cm8[b, h] = [128, PTS_CM] channel-major half-tiles
    cmd = nc.dram_tensor("cm8", [BL, 2, P, PTS_CM], f8, kind="ExternalInput")
    pmd = nc.dram_tensor("pm8", [BL, P, PM_F], f8, kind="ExternalInput")
    ones8d = nc.dram_tensor("ones8", [P, 1], f8, kind="ExternalInput")
    wp16d = nc.dram_tensor("wp16", [P, W16_F], f16, kind="ExternalInput")
    wrowd = nc.dram_tensor("wrow", [1, WROW_F], f16, kind="ExternalInput")
    b3twod = nc.dram_tensor("b3two", [BL, CH], f32, kind="ExternalInput")
    out_rows = nc.dram_tensor("out_rows", [BL, CH], f32, kind="ExternalOutput")

    with TileContext(nc) as tc:
        with (
            tc.tile_pool(name="w", bufs=1) as wpool,
            tc.tile_pool(name="cmio", bufs=2 * BL) as cmpool,
            tc.tile_pool(name="pmio", bufs=BL) as pmpool,
            tc.tile_pool(name="junk", bufs=2) as jpool,
            tc.tile_pool(name="small", bufs=1) as spool,
            tc.tile_pool(name="sred", bufs=BL, space="PSUM") as srpool,
            tc.tile_pool(name="mt", bufs=1, space="PSUM") as mtpool,
            tc.tile_pool(name="mm", bufs=2, space="PSUM") as mmpool,
            tc.tile_pool(name="orp", bufs=1, space="PSUM") as orpool,
        ):
            # tiny early weights ride first on the sync queue
            ones8 = wpool.tile([P, 1], f8, tag="ones8")
            nc.sync.dma_start(out=ones8[:, :], in_=ones8d[:, :])
            wrow = wpool.tile([1, WROW_F], f16, tag="wrow")
            nc.sync.dma_start(out=wrow[:, :], in_=wrowd[:, :])
            one16 = wrow[0:1, OFF_ONES:OFF_ONES + 1]
            ones2 = wrow[0:1, OFF_ONES:OFF_ONES + BL]

            # lidar chunks: pm_b0, cm_b0 halves, pm_b1, cm_b1 halves
            pmt, cmt = [], {}
            for b in range(BL):
                t = pmpool.tile([P, PM_F], f8, tag="pm")
                nc.sync.dma_start(out=t[:, :], in_=pmd[b, :, :])
                pmt.append(t)
                for h in range(2):
                    c = cmpool.tile([P, PTS_CM], f8, tag="cm")
                    nc.sync.dma_start(out=c[:, :], in_=cmd[b, h, :, :])
                    cmt[(b, h)] = c
            # remaining weights after the lidar stream
            wp16 = wpool.tile([P, W16_F], f16, tag="wp16")
            nc.sync.dma_start(out=wp16[:, :], in_=wp16d[:, :])
            b3sb = wpool.tile([BL, CH], f32, tag="b3sb")
            nc.sync.dma_start(out=b3sb[:, :], in_=b3twod[:, :])

            # ---- point-major: ones^T @ tile chains on TensorE ----
            sred = []
            for b in range(BL):
                sr = srpool.tile([1, MM_F], f32, tag="sred")
                nmm = PM_F // MM_F
                for j in range(nmm):
                    nc.tensor.matmul(sr[:, :], lhsT=ones8[:, :],
                                     rhs=pmt[b][:, j * MM_F:(j + 1) * MM_F],
                                     start=(j == 0), stop=(j == nmm - 1))
                sred.append(sr)

            # ---- channel-major: DVE / ACT / GPSIMD per half-tile ----
            # partials: col = 6b + 3h + k, k in {DVE, ACT, fold}
            S = spool.tile([P, 6 * BL], f32, tag="S")
            for b in range(BL):
                for h in range(2):
                    tin = cmt[(b, h)]
                    c0 = 6 * b + 3 * h
                    nc.vector.reduce_sum(
                        out=S[:, c0:c0 + 1],
                        in_=tin[:, 0:DVE_N], axis=Ax.X)
                    ja = jpool.tile([P, ACT_N], f16, tag="ja")
                    nc.scalar.activation(
                        ja[:, :], tin[:, DVE_N:DVE_N + ACT_N],
                        Act.Copy, accum_out=S[:, c0 + 1:c0 + 2])
                    jg = jpool.tile([P, GH], f16, tag="jg")
                    base = DVE_N + ACT_N
                    nc.gpsimd.tensor_add(
                        out=jg[:, :],
                        in0=tin[:, base:base + GH],
                        in1=tin[:, base + GH:base + 2 * GH])
                    nc.vector.reduce_sum(
                        out=S[:, c0 + 2:c0 + 3],
                        in_=jg[:, :], axis=Ax.X)

            # pm fold [1,512] -> [1,256] fp16 and transpose to columns
            # mtp columns in (b, h) order: [b0A, b0B, b1A, b1B]
            mtp = mtpool.tile([P, 2 * BL], f32, tag="mtp")
            for b in range(BL):
                s512 = spool.tile([1, MM_F], f32, tag=f"s512{b}")
                nc.scalar.copy(out=s512[:, :], in_=sred[b][:, :])
                s16 = spool.tile([1, CH], f16, tag=f"s16{b}")
                nc.vector.tensor_add(out=s16[:, :], in0=s512[0:1, 0:CH],
                                     in1=s512[0:1, CH:MM_F])
                for h in range(2):
                    nc.tensor.matmul(mtp[:, 2 * b + h:2 * b + h + 1],
                                     lhsT=s16[0:1, h * P:(h + 1) * P],
                                     rhs=one16, start=True, stop=True,
                                     skip_group_check=True)

            # assemble means: S is (b, kind, h)-ordered; present it to the
            # reducer as (b h) groups of 3 strided cols, writing m32 in
            # (h, b) memory order to match mtp / the MLP column layout.
            S3 = S[:, :].rearrange("p (g k) -> p g k", k=3)
            m32 = spool.tile([P, 2 * BL], f32, tag="m32")
            nc.vector.reduce_sum(out=m32[:, :], in_=S3, axis=Ax.X)
            msum = spool.tile([P, 2 * BL], f32, tag="msum")
            nc.vector.tensor_add(out=msum[:, :], in0=m32[:, :], in1=mtp[:, :])
            m16 = spool.tile([P, 2 * BL], f16, tag="m16")
            nc.vector.tensor_scalar_mul(m16[:, :], msum[:, :], float(1.0 / NPTS))
            # halves as strided views: mh[:, h, :] has the BL batch columns
            m16v = m16[:, :].rearrange("p (b h) -> p h b", b=BL, h=2)

            def leaky(zp, tag):
                z01 = spool.tile([P, BL], f16, tag=f"z{tag}")
                nc.vector.tensor_scalar_mul(z01[:, :], zp[:, :], 0.01)
                h = spool.tile([P, BL], f16, tag=f"h{tag}")
                nc.vector.tensor_max(h[:, :], zp[:, :], z01[:, :])
                return h

            # h1 = leaky(W1.T @ m + b1 ⊗ ones)
            h1p = mmpool.tile([P, BL], f32, tag="mm")
            nc.tensor.matmul(h1p[:, :], lhsT=wp16[:, OFF_W1A:OFF_W1A + P],
                             rhs=m16v[:, 0:1, :], start=True, stop=False)
            nc.tensor.matmul(h1p[:, :], lhsT=wp16[:, OFF_W1B:OFF_W1B + P],
                             rhs=m16v[:, 1:2, :], start=False, stop=False)
            nc.tensor.matmul(h1p[:, :], lhsT=wrow[0:1, OFF_B1:OFF_B1 + P],
                             rhs=ones2, start=False, stop=True)
            h1 = leaky(h1p, "1")

            # h2 = leaky(wo2.T @ h1 + b2 ⊗ ones)
            h2p = mmpool.tile([P, BL], f32, tag="mm")
            nc.tensor.matmul(h2p[:, :], lhsT=wp16[:, OFF_WO2:OFF_WO2 + P],
                             rhs=h1[:, :], start=True, stop=False)
            nc.tensor.matmul(h2p[:, :], lhsT=wrow[0:1, OFF_B2:OFF_B2 + P],
                             rhs=ones2, start=False, stop=True)
            h2 = leaky(h2p, "2")

            # out rows = h2.T @ wo3 + b3  -> [2, 256]
            orp = orpool.tile([BL, CH], f32, tag="orp")
            nc.tensor.matmul(orp[:, :], lhsT=h2[:, :],
                             rhs=wp16[:, OFF_WO3:OFF_WO3 + CH],
                             start=True, stop=True)
            orow = spool.tile([BL, CH], f32, tag="orow")
            nc.vector.tensor_add(out=orow[:, :], in0=orp[:, :], in1=b3sb[:, :])
            nc.sync.dma_start(out=out_rows[:, :], in_=orow[:, :])

    nc.compile()
    return nc


def _pack_weights(inputs):
    wv = np.asarray(inputs["wv"], np.float64)
    wo1 = np.asarray(inputs["wo1"], np.float64)
    W1 = (wv @ wo1)                            # [256, 128], linear chain

    wp16 = np.zeros((P, W16_F), np.float16)
    wp16[:, OFF_W1A:OFF_W1A + P] = W1[0:128, :]
    wp16[:, OFF_W1B:OFF_W1B + P] = W1[128:256, :]
    wp16[:, OFF_WO2:OFF_WO2 + P] = np.asarray(inputs["wo2"], np.float32)
    wp16[:, OFF_WO3:OFF_WO3 + CH] = np.asarray(inputs["wo3"], np.float32)

    wrow = np.zeros((1, WROW_F), np.float16)
    wrow[0, OFF_B1:OFF_B1 + P] = np.asarray(inputs["b1"], np.float32)
    wrow[0, OFF_B2:OFF_B2 + P] = np.asarray(inputs["b2"], np.float32)
    wrow[0, OFF_ONES:OFF_ONES + BL] = 1.0

    b3two = np.broadcast_to(
        np.asarray(inputs["b3"], np.float32).reshape(1, CH), (BL, CH)).copy()
    return wp16, wrow, b3two


def kernel(**inputs):
    import ml_dtypes
    from concourse.bass_utils import run_bass_kernel_spmd

    if "nc" not in _CACHE:
        _CACHE["nc"] = _build_program()
    nc = _CACHE["nc"]

    f8 = ml_dtypes.float8_e3m4
    lid = np.asarray(inputs["lidar"], dtype=np.float32).reshape(
        N_CORES, BL, NPTS, CH)
    # point-major region: [c, b, PTS_PM, 256] -> [c, b, 128, PM_F]
    pm8 = np.ascontiguousarray(lid[:, :, :PTS_PM, :]).astype(f8).reshape(
        N_CORES, BL, P, PM_F)
    # channel-major region: [c, b, PTS_CM, 256] -> [c, b, 256, PTS_CM]
    # -> [c, b, 2(half), 128, PTS_CM]
    cm8 = np.ascontiguousarray(
        lid[:, :, PTS_PM:, :].transpose(0, 1, 3, 2)).astype(f8).reshape(
        N_CORES, BL, 2, P, PTS_CM)

    ones8 = np.ones((P, 1), f8)
    wp16, wrow, b3two = _pack_weights(inputs)

    in_maps = [
        {"cm8": cm8[i], "pm8": pm8[i], "ones8": ones8,
         "wp16": wp16, "wrow": wrow, "b3two": b3two}
        for i in range(N_CORES)
    ]
    res = run_bass_kernel_spmd(nc, in_maps, list(range(N_CORES)),
                               **_CACHE.get("run_kwargs", {}))
    _CACHE["last_results"] = res
    out = np.concatenate([res.results[i]["out_rows"] for i in range(N_CORES)], axis=0)
    return np.ascontiguousarray(out, dtype=np.float32)
